# revision 8
# baseline (speedup 1.0000x reference)
"""3D-RoPE multi-head attention on 8 TRN2 NeuronCores.

Sharding: data-parallel over batch (4) x tensor-parallel over head-halves (2)
= 8 shards. Core c handles batch c//2, heads (c%2)*8 .. (c%2)*8+8.
Each core computes its 8 heads' attention plus the partial output projection
(rows of W_proj for its heads); host sums the two partials per batch + bias.

Device algorithm (per core), all matmuls in float32r (fp32 storage, full PE
rate, ~1e-4 rel err):
  qkT[col, tok] = W_qkv_padT-stationary matmul vs X^T   (head-dim on partitions)
  rope via elementwise cos/sin + a 128x128 permutation matmul
  S^T[m, q]     = K^T-stationary matmul (keys on psum partitions)
  P~^T          = exp(S^T / sqrt(48)) on ScalarE, psum->sbuf, no max-subtraction
  O^T unnorm    = V'-stationary matmul over P~^T; V' carries a ones-column so
                  row 48/112 of the accumulator is the softmax denominator
  normalize via reciprocal + ones-outer-product broadcast matmul
  Y partial     = O^T-stationary matmul vs padded W_proj rows
Heads are processed in pairs packed at partition offsets 0 and 64 (row/col
tile_position packing) to recover PE utilization at head_dim=48.
"""

import sys

sys.path.insert(0, "/opt/trn_rl_repo")

import numpy as np

import concourse.bass as bass  # noqa: F401  (import order: bass before tile)
import concourse.mybir as mybir
import concourse.tile as tile
from concourse import bacc
from concourse.bass_utils import run_bass_kernel_spmd

# Problem constants (hardcoded; kernel.py must be self-contained).
B, N, DIM = 4, 1568, 768
NHEAD, HD = 16, 48
AXIS = 16           # head-dim per spatial axis
HALF = 8            # rotation pairs per axis
ROPE_BASE = 10000.0
NH_LOC = 8          # heads per core
PAIRS = 4           # head pairs per core
SCALE = 1.0 / np.sqrt(HD)

MT = [128] * 12 + [32]                     # key/token tile sizes, 13 tiles
CHUNKS = [(0, 512), (512, 512), (1024, 512), (1536, 32)]
GROUPS = [[0, 1, 2], [3, 4, 5], [6, 7, 8], [9, 10, 11], [12]]
KT = 6                                     # 768 / 128 contraction tiles

F32 = mybir.dt.float32
F32R = mybir.dt.float32r
BF16 = mybir.dt.bfloat16
MULT = mybir.AluOpType.mult
ADD = mybir.AluOpType.add
EXP = mybir.ActivationFunctionType.Exp

_NC_CACHE = None


def _build_nc():
    nc = bacc.Bacc(None, target_bir_lowering=False, debug=False)
    with tile.TileContext(nc) as tc:
        xt_d = nc.dram_tensor("xt", [DIM, N], F32R, kind="ExternalInput")
        wqk_d = nc.dram_tensor("wqk", [DIM, 1024], F32R, kind="ExternalInput")
        wv_d = nc.dram_tensor("wv", [DIM, 384], F32R, kind="ExternalInput")
        wp_d = nc.dram_tensor("wp", [512, DIM], F32R, kind="ExternalInput")
        cos_d = nc.dram_tensor("cosp", [128, N], F32, kind="ExternalInput")
        sin_d = nc.dram_tensor("sinp", [128, N], F32, kind="ExternalInput")
        perm_d = nc.dram_tensor("perm", [128, 128], F32R, kind="ExternalInput")
        ones_d = nc.dram_tensor("ones64", [128, 64], BF16, kind="ExternalInput")
        y_d = nc.dram_tensor("y", [N, DIM], F32, kind="ExternalOutput")

        with tc.tile_pool(name="sb", bufs=1) as sb, \
             tc.tile_pool(name="ps", bufs=1, space="PSUM") as ps:
            xt = []
            wqk = []
            wv = []
            for k in range(KT):
                t = sb.tile([128, N], F32R, tag=f"xt{k}")
                nc.sync.dma_start(t[:], xt_d[k * 128:(k + 1) * 128, :])
                xt.append(t)
                t = sb.tile([128, 1024], F32R, tag=f"wqk{k}")
                nc.sync.dma_start(t[:], wqk_d[k * 128:(k + 1) * 128, :])
                wqk.append(t)
                t = sb.tile([128, 384], F32R, tag=f"wv{k}")
                nc.sync.dma_start(t[:], wv_d[k * 128:(k + 1) * 128, :])
                wv.append(t)
            wp = []
            for p in range(PAIRS):
                t = sb.tile([128, DIM], F32R, tag=f"wp{p}")
                nc.sync.dma_start(t[:], wp_d[p * 128:(p + 1) * 128, :])
                wp.append(t)
            perm_t = sb.tile([128, 128], F32R, tag="perm")
            nc.sync.dma_start(perm_t[:], perm_d[:])
            ones_t = sb.tile([128, 64], BF16, tag="ones64")
            nc.sync.dma_start(ones_t[:], ones_d[:])

            ot = [sb.tile([128, N], F32R, tag=f"ot{p}", name=f"ot{p}")
                  for p in range(PAIRS)]

            def emit_rope(pt_i):
                """Project one q- or k- head-pair column tile and apply rope.

                pt_i 0-3 = q pairs, 4-7 = k pairs. Returns [128, N] f32r tile
                (head even at partitions 0-47, head odd at 64-111, pads 0).
                """
                rot = sb.tile([128, N], F32R, tag="qkrot", bufs=4, name=f"rot{pt_i}")
                for off, cs in CHUNKS:
                    cos_t = sb.tile([128, 512], F32, tag="cos", bufs=2, name="cos_t")
                    nc.sync.dma_start(cos_t[:, :cs], cos_d[:, off:off + cs])
                    sin_t = sb.tile([128, 512], F32, tag="sin", bufs=2, name="sin_t")
                    nc.sync.dma_start(sin_t[:, :cs], sin_d[:, off:off + cs])
                    qk_ps = ps.tile([128, 512], F32, tag="b1", bufs=2, name="qk_ps")
                    for k in range(KT):
                        nc.tensor.matmul(
                            qk_ps[:, :cs],
                            wqk[k][:, pt_i * 128:(pt_i + 1) * 128],
                            xt[k][:, off:off + cs],
                            start=(k == 0), stop=(k == KT - 1))
                    raw = sb.tile([128, 512], F32, tag="raw", bufs=2, name="raw")
                    nc.vector.tensor_copy(raw[:, :cs], qk_ps[:, :cs])
                    u = sb.tile([128, 512], F32R, tag="u", bufs=2, name="u")
                    nc.vector.tensor_tensor(u[:, :cs], raw[:, :cs],
                                            sin_t[:, :cs], MULT)
                    nc.vector.tensor_tensor(raw[:, :cs], raw[:, :cs],
                                            cos_t[:, :cs], MULT)
                    pp = ps.tile([128, 512], F32, tag="b1", bufs=2, name="pp")
                    nc.tensor.matmul(pp[:, :cs], perm_t[:], u[:, :cs],
                                     start=True, stop=True)
                    nc.vector.tensor_tensor(rot[:, off:off + cs], pp[:, :cs],
                                            raw[:, :cs], ADD)
                return rot

            def emit_v():
                vt = []
                for m in range(13):
                    mt = MT[m]
                    v_ps = ps.tile([128, 512], F32, tag="b1", bufs=2, name="v_ps")
                    for k in range(KT):
                        nc.tensor.matmul(
                            v_ps[:mt, :384],
                            xt[k][:, m * 128:m * 128 + mt],
                            wv[k][:],
                            start=(k == 0), stop=(k == KT - 1))
                    t = sb.tile([128, 8 * 49], BF16, tag=f"v{m}", name=f"v{m}")
                    dst = t[:mt, :].rearrange("p (h w) -> p h w", w=49)
                    src = v_ps[:mt, :384].rearrange("p (h w) -> p h w", w=48)
                    nc.vector.tensor_copy(dst[:, :, 1:49], src)
                    ones_src = ones_t[:mt, 1:9].rearrange("p (h o) -> p h o", o=1)
                    nc.vector.tensor_copy(dst[:, :, 0:1], ones_src)
                    vt.append(t)
                return vt

            def emit_attn(p, qrot, krot, v):
                otp = ot[p]
                for off, cs in CHUNKS:
                    av = ps.tile([128, 512], F32, tag="b1", bufs=2, name="av")
                    for ms in GROUPS:
                        s_list = []
                        for h in (0, 1):
                            hoff = h * 64
                            s_ps = ps.tile([128, 3, 512], F32, tag="s", bufs=2, name="s_ps")
                            for gi, m in enumerate(ms):
                                mt = MT[m]
                                nc.tensor.matmul(
                                    s_ps[:mt, gi, :cs],
                                    krot[hoff:hoff + 48, m * 128:m * 128 + mt],
                                    qrot[hoff:hoff + 48, off:off + cs],
                                    start=True, stop=True,
                                    tile_position=(hoff, 0))
                            s_list.append(s_ps)
                        pt_list = []
                        for h in (0, 1):
                            mtg = MT[ms[0]]
                            pt_t = sb.tile([128, 3, 512], BF16, tag="pt",
                                           bufs=2, name="pt_t")
                            nc.scalar.activation(
                                pt_t[:mtg, 0:len(ms), :cs],
                                s_list[h][:mtg, 0:len(ms), :cs],
                                EXP, scale=float(SCALE))
                            pt_list.append(pt_t)
                        for h in (0, 1):
                            hoff = h * 64
                            hloc = 2 * p + h
                            for gi, m in enumerate(ms):
                                mt = MT[m]
                                nc.tensor.matmul(
                                    av[hoff:hoff + 49, :cs],
                                    v[m][:mt, hloc * 49:hloc * 49 + 49],
                                    pt_list[h][:mt, gi, :cs],
                                    start=(m == 0), stop=(m == 12),
                                    tile_position=(0, hoff))
                    nc.vector.tensor_copy(otp[:, off:off + cs], av[:, :cs])
                # denominators -> reciprocal in place (rows 48 and 112)
                with nc.allow_low_precision(reason="softmax denom in f32r"):
                    for row in (0, 64):
                        nc.vector.reciprocal(otp[row:row + 1, :],
                                             otp[row:row + 1, :])
                rcpb = sb.tile([128, N], BF16, tag="rcpb", name="rcpb")
                for row in (0, 64):
                    nc.vector.tensor_copy(rcpb[row:row + 1, :],
                                          otp[row:row + 1, :])
                for off, cs in CHUNKS:
                    db = ps.tile([128, 512], F32, tag="b1", bufs=2, name="db")
                    nc.tensor.matmul(db[0:64, :cs], ones_t[0:1, :],
                                     rcpb[0:1, off:off + cs],
                                     start=True, stop=True,
                                     tile_position=(0, 0))
                    nc.tensor.matmul(db[64:128, :cs], ones_t[64:65, :],
                                     rcpb[64:65, off:off + cs],
                                     start=True, stop=True,
                                     tile_position=(64, 64))
                    nc.vector.tensor_tensor(otp[:, off:off + cs],
                                            otp[:, off:off + cs],
                                            db[:, :cs], MULT)

            # Emission order = scheduling priority: pair 0's q/k first so
            # attention starts ASAP; later pairs' projections overlap it.
            rot_tiles = {}
            rot_tiles[0] = emit_rope(0)
            rot_tiles[4] = emit_rope(4)
            v = emit_v()
            for p in range(PAIRS):
                if p + 1 < PAIRS:
                    rot_tiles[p + 1] = emit_rope(p + 1)
                    rot_tiles[p + 5] = emit_rope(p + 5)
                emit_attn(p, rot_tiles[p], rot_tiles[p + 4], v)

            # output projection: Y[tok, :] = sum_p ot[p][:, tok].T @ wp[p]
            for tt in range(13):
                mt = MT[tt]
                y_t = sb.tile([128, DIM], F32, tag="y", bufs=2, name="y_t")
                for half in (0, 1):
                    y_ps = ps.tile([128, 512], F32, tag="b1", bufs=2, name="y_ps")
                    for p in range(PAIRS):
                        nc.tensor.matmul(
                            y_ps[:mt, :384],
                            ot[p][:, tt * 128:tt * 128 + mt],
                            wp[p][:, half * 384:half * 384 + 384],
                            start=(p == 0), stop=(p == PAIRS - 1))
                    nc.vector.tensor_copy(
                        y_t[:mt, half * 384:half * 384 + 384],
                        y_ps[:mt, :384])
                nc.sync.dma_start(y_d[tt * 128:tt * 128 + mt, :], y_t[:mt, :])
    nc.compile()
    return nc


def _rope_tables():
    """cos/sin patterns in pair-padded [128, N] layout + perm matrix.

    rope(t)[d] = t[d]*cos48[d] + t[partner(d)]*sinsgn48[d]
    implemented as rot = t*cos + Perm(t*s2), s2[e] = sinsgn48[partner(e)].
    """
    t, y, xg = np.meshgrid(np.arange(8), np.arange(14), np.arange(14),
                           indexing="ij")
    pos = np.stack([t.ravel(), y.ravel(), xg.ravel()], axis=-1).astype(np.float64)
    inv_freq = ROPE_BASE ** (-np.arange(HALF, dtype=np.float64) / HALF)
    ang = pos[:, :, None] * inv_freq[None, None, :]          # [N, 3, 8]
    cos48 = np.zeros((HD, N), np.float32)
    sinsgn48 = np.zeros((HD, N), np.float32)
    partner = np.zeros(HD, np.int64)
    for d in range(HD):
        axis, jj = d // AXIS, d % AXIS
        j = jj % HALF
        cos48[d] = np.cos(ang[:, axis, j])
        sinsgn48[d] = (-1.0 if jj < HALF else 1.0) * np.sin(ang[:, axis, j])
        partner[d] = axis * AXIS + (jj + HALF) % AXIS
    s2_48 = sinsgn48[partner]                                # [48, N]
    cosp = np.zeros((128, N), np.float32)
    s2p = np.zeros((128, N), np.float32)
    for base in (0, 64):
        cosp[base:base + HD] = cos48
        s2p[base:base + HD] = s2_48
    perm = np.zeros((128, 128), np.float32)
    for base in (0, 64):
        for d in range(HD):
            perm[base + partner[d], base + d] = 1.0
    return cosp, s2p, perm


def _shards(x, pos, W_qkv, W_proj):
    cosp, s2p, perm = _rope_tables()
    import ml_dtypes
    ones64 = np.zeros((128, 64), ml_dtypes.bfloat16)
    ones64[:, 1:49] = 1.0
    in_maps = []
    for c in range(8):
        b, hg = c // 2, c % 2
        heads = [hg * NH_LOC + i for i in range(NH_LOC)]
        wqk = np.zeros((DIM, 1024), np.float32)
        wv = np.zeros((DIM, 384), np.float32)
        wp = np.zeros((512, DIM), np.float32)
        for i, h in enumerate(heads):
            wqk[:, i * 64:i * 64 + HD] = W_qkv[:, h * HD:(h + 1) * HD]
            wqk[:, 512 + i * 64:512 + i * 64 + HD] = \
                W_qkv[:, DIM + h * HD:DIM + (h + 1) * HD]
            wv[:, i * HD:(i + 1) * HD] = \
                W_qkv[:, 2 * DIM + h * HD:2 * DIM + (h + 1) * HD]
            base = (i // 2) * 128 + (i % 2) * 64
            wp[base + 1:base + 1 + HD, :] = W_proj[h * HD:(h + 1) * HD, :]
        in_maps.append({
            "xt": np.ascontiguousarray(x[b].T).astype(np.float32),
            "wqk": wqk, "wv": wv, "wp": wp,
            "cosp": cosp, "sinp": s2p, "perm": perm, "ones64": ones64,
        })
    return in_maps


def kernel(x, pos, W_qkv, W_proj, b_proj):
    global _NC_CACHE
    x = np.asarray(x, np.float32)
    W_qkv = np.asarray(W_qkv, np.float32)
    W_proj = np.asarray(W_proj, np.float32)
    b_proj = np.asarray(b_proj, np.float32)
    if _NC_CACHE is None:
        _NC_CACHE = _build_nc()
    in_maps = _shards(x, pos, W_qkv, W_proj)
    res = run_bass_kernel_spmd(_NC_CACHE, in_maps, core_ids=list(range(8)))
    out = np.empty((B, N, DIM), np.float32)
    for b in range(B):
        out[b] = res.results[2 * b]["y"] + res.results[2 * b + 1]["y"] \
            + b_proj[None, :]
    return out
